# revision 11
# baseline (speedup 1.0000x reference)
"""Trainium2 Bass kernel for the Gaussian-mixture image renderer (nn_MoE).

Math (reformulated from the reference nn.Module):
  out[a, h, w] = sum_k w[a,k]*e_k / max(sum_k e_k, 1e-8),
  e_k = exp(q_ak(x, y)), x = lin[h], y = lin[w], lin = linspace(0,1,256)
  q_ak is a quadratic polynomial in (x, y) whose 6 monomial coefficients are
  computed on the host from mu/L/softmax(w) (tiny: 24*16*6 floats).

Device strategy (8 cores, data-parallel over pixels):
  Each core processes all 24 images for 8192 pixels (1/8 of the image).
  Images are processed in 3 groups of 8; within a group the 128 SBUF
  partitions hold all (image, gaussian) pairs (8*16 = 128).
  Per 512-pixel chunk:
    1. TensorE: q = coef(6,128).T @ basis(6,512)      -> PSUM (128,512)
    2. ScalarE: e = exp(q)                             PSUM -> SBUF
    3. TensorE: two reduction matmuls over the partition dim with
       block-diagonal ones / softmax-weight matrices (M=32, col-tiled via
       tile_position so 4 chunks pack into one (128,512) PSUM tile)
    4. DVE: y = wsum * reciprocal(sum)                 -> SBUF -> DMA out
"""

import sys

if "/opt/trn_rl_repo" not in sys.path:
    sys.path.insert(0, "/opt/trn_rl_repo")

from contextlib import ExitStack

import ml_dtypes
import numpy as np

K = 16
A = 24
H = W = 256
PIX = H * W
N_CORES = 8
PPC = PIX // N_CORES  # pixels per core = 8192
NG = 3  # image groups of 8
F32 = None  # set after mybir import


# ----------------------------------------------------------------------------
# Host-side parameter preprocessing
# ----------------------------------------------------------------------------

def _softmax_np(x):
    x = x.astype(np.float32)
    m = x.max(axis=-1, keepdims=True)
    e = np.exp(x - m)
    return (e / e.sum(axis=-1, keepdims=True)).astype(np.float32)


def _compute_coef_w(params):
    """params (8,3,112) -> coef (A, K, 6) fp32 (basis order [1,x,y,x2,xy,y2]),
    w (A, K) fp32."""
    p = np.asarray(params, dtype=np.float32).reshape(A, 7 * K)
    mu0 = p[:, :K]
    mu1 = p[:, K : 2 * K]
    w = _softmax_np(p[:, 2 * K : 3 * K])
    raw = p[:, 3 * K : 7 * K].reshape(A, K, 2, 2)
    l00 = raw[:, :, 0, 0]
    l10 = raw[:, :, 1, 0]
    l11 = raw[:, :, 1, 1]
    s0 = l00 * l00 + l00 * l10
    s1 = l00 * l10 + l10 * l10 + l11 * l11
    s01 = s0 + s1
    c00 = -0.5 * (s0 * mu0 * mu0 + s01 * mu0 * mu1 + s1 * mu1 * mu1)
    c10 = 0.5 * (2.0 * s0 * mu0 + s01 * mu1)
    c01 = 0.5 * (s01 * mu0 + 2.0 * s1 * mu1)
    c20 = -0.5 * s0
    c11 = -0.5 * s01
    c02 = -0.5 * s1
    coef = np.stack([c00, c10, c01, c20, c11, c02], axis=-1).astype(np.float32)
    return coef, w.astype(np.float32)


def _compute_basis():
    """(6, PIX) fp32 monomial basis; pixel n = h*256 + w, x=lin[h], y=lin[w]."""
    lin = np.linspace(0.0, 1.0, 256, dtype=np.float32)
    x = np.repeat(lin, W)
    y = np.tile(lin, H)
    return np.stack([np.ones_like(x), x, y, x * x, x * y, y * y], axis=0).astype(
        np.float32
    )


def _host_inputs(params):
    """Build the per-core input maps."""
    coef, w = _compute_coef_w(params)  # (24,16,6), (24,16)

    # coef_all (6, 128*NG): group g, partition p = 16*j + k (j: image slot)
    coef_all = np.zeros((6, 128 * NG), np.float32)
    for g in range(NG):
        for j in range(8):
            a = 8 * g + j
            # (K, 6) -> columns 128*g + 16*j + k
            coef_all[:, 128 * g + 16 * j : 128 * g + 16 * j + K] = coef[a].T

    # red_ones (128, 32): col m<8 -> ones over partitions of image m;
    # cols 8..31 -> all-ones (benign padding so every PSUM row is defined)
    red_ones = np.zeros((128, 32), np.float32)
    for j in range(8):
        red_ones[16 * j : 16 * j + K, j] = 1.0
    red_ones[:, 8:] = 1.0

    # red_w (128, 32*NG): same masks but with softmax weights
    red_w = np.zeros((128, 32 * NG), np.float32)
    for g in range(NG):
        for j in range(8):
            red_w[16 * j : 16 * j + K, 32 * g + j] = w[8 * g + j]
        red_w[:, 32 * g + 8 : 32 * g + 32] = 1.0

    basis = _compute_basis()  # (6, PIX)

    in_maps = []
    for c in range(N_CORES):
        in_maps.append(
            {
                "basis": np.ascontiguousarray(basis[:, c * PPC : (c + 1) * PPC]),
                "coef": coef_all,
                "red_ones": red_ones.astype(ml_dtypes.bfloat16),
                "red_w": red_w.astype(ml_dtypes.bfloat16),
            }
        )
    return in_maps


# ----------------------------------------------------------------------------
# Bass kernel
# ----------------------------------------------------------------------------

_NC_CACHE = {}


def _build_nc():
    if "nc" in _NC_CACHE:
        return _NC_CACHE["nc"]

    import concourse.bacc as bacc
    import concourse.mybir as mybir
    import concourse.tile as tile

    f32 = mybir.dt.float32
    f32r = mybir.dt.float32r
    bf16 = mybir.dt.bfloat16
    nc = bacc.Bacc("TRN2", target_bir_lowering=False, debug=False,
                   enable_asserts=False)

    # float32r: single-pass PE matmul (~tf32 precision) vs 2-pass fp32.
    # Verified final output error vs reference: max abs 1.7e-4.
    basis_d = nc.dram_tensor("basis", (6, PPC), f32r, kind="ExternalInput").ap()
    coef_d = nc.dram_tensor("coef", (6, 128 * NG), f32r, kind="ExternalInput").ap()
    ones_d = nc.dram_tensor("red_ones", (128, 32), bf16, kind="ExternalInput").ap()
    wred_d = nc.dram_tensor("red_w", (128, 32 * NG), bf16, kind="ExternalInput").ap()
    out_d = nc.dram_tensor("out", (NG, 8, PPC), f32, kind="ExternalOutput").ap()

    EXP = mybir.ActivationFunctionType.Exp

    with tile.TileContext(nc) as tc:
        with ExitStack() as ctx:
            const_pool = ctx.enter_context(tc.tile_pool(name="const", bufs=1))
            pe_pool = ctx.enter_context(
                tc.tile_pool(name="pe", bufs=2, space="PSUM")
            )
            ps_pool = ctx.enter_context(
                tc.tile_pool(name="ps", bufs=2, space="PSUM")
            )
            pw_pool = ctx.enter_context(
                tc.tile_pool(name="pw", bufs=2, space="PSUM")
            )
            e_pool = ctx.enter_context(tc.tile_pool(name="e", bufs=4))
            y_pool = ctx.enter_context(tc.tile_pool(name="y", bufs=2))
            r_pool = ctx.enter_context(tc.tile_pool(name="r", bufs=2))

            coef_sb = const_pool.tile([6, 128 * NG], f32r)
            nc.sync.dma_start(coef_sb[:], coef_d[:])
            ones_sb = const_pool.tile([128, 32], bf16)
            nc.sync.dma_start(ones_sb[:], ones_d[:])
            wred_sb = const_pool.tile([128, 32 * NG], bf16)
            nc.sync.dma_start(wred_sb[:], wred_d[:])
            # basis split into slices so the first matmuls can start early
            basis_sb = const_pool.tile([6, PPC], f32r)
            for s in range(8):
                nc.sync.dma_start(
                    basis_sb[:, 1024 * s : 1024 * (s + 1)],
                    basis_d[:, 1024 * s : 1024 * (s + 1)],
                )

            # out viewed as [g, hh, cpart, j, qq, col];
            # pixel = 4096*hh + 2048*qq + 512*cpart + col
            out6 = out_d.rearrange(
                "g j (hh qq cpart col) -> g hh cpart j qq col",
                hh=2, qq=2, cpart=4, col=512,
            )

            for g in range(NG):
                coef_g = coef_sb[:, 128 * g : 128 * (g + 1)]
                w_g = wred_sb[:, 32 * g : 32 * (g + 1)]
                for half in range(2):
                    y_half = y_pool.tile([128, 1024], f32)
                    for qq in range(2):
                        quarter = 2 * half + qq
                        base = 2048 * quarter
                        psum_s = ps_pool.tile([128, 512], f32)
                        psum_w = pw_pool.tile([128, 512], f32)
                        for t in range(2):
                            pe = pe_pool.tile([128, 1024], f32)
                            off = base + 1024 * t
                            nc.tensor.matmul(
                                pe[:, 0:512], coef_g,
                                basis_sb[:, off : off + 512],
                                start=True, stop=True,
                            )
                            nc.tensor.matmul(
                                pe[:, 512:1024], coef_g,
                                basis_sb[:, off + 512 : off + 1024],
                                start=True, stop=True,
                            )
                            e = e_pool.tile([128, 1024], bf16)
                            nc.scalar.activation(e[:], pe[:], EXP)
                            for u in range(2):
                                c = 2 * t + u
                                rhs = e[:, 512 * u : 512 * (u + 1)]
                                nc.tensor.matmul(
                                    psum_s[32 * c : 32 * (c + 1), :],
                                    ones_sb[:], rhs,
                                    start=True, stop=True,
                                    tile_position=(0, 32 * c),
                                )
                                nc.tensor.matmul(
                                    psum_w[32 * c : 32 * (c + 1), :],
                                    w_g, rhs,
                                    start=True, stop=True,
                                    tile_position=(0, 32 * c),
                                )
                        r = r_pool.tile([128, 512], f32)
                        nc.vector.reciprocal_approx_fast(r[:], psum_s[:])
                        nc.vector.tensor_mul(
                            y_half[:, 512 * qq : 512 * (qq + 1)],
                            psum_w[:], r[:],
                        )
                    for c in range(4):
                        src = y_half[32 * c : 32 * c + 8, :].rearrange(
                            "j (qq col) -> j qq col", qq=2
                        )
                        nc.sync.dma_start(out6[g, half, c], src)

    nc.compile()
    _NC_CACHE["nc"] = nc
    return nc


def _run(in_maps, **spmd_kwargs):
    from concourse.bass_utils import run_bass_kernel_spmd

    nc = _build_nc()
    return run_bass_kernel_spmd(
        nc, in_maps, core_ids=list(range(N_CORES)), **spmd_kwargs
    )


def _assemble(results):
    """results: list of 8 dicts with 'out' (NG, 8, PPC) -> (8,3,256,256)."""
    full = np.empty((A, PIX), dtype=np.float32)
    for c, res in enumerate(results):
        full[:, c * PPC : (c + 1) * PPC] = res["out"].reshape(A, PPC)
    return full.reshape(8, 3, H, W)


def kernel(params, height, width):
    assert int(height) == H and int(width) == W
    in_maps = _host_inputs(params)
    res = _run(in_maps)
    return _assemble(res.results)


if __name__ == "__main__":
    params = np.random.RandomState(0).randn(8, 3, 7 * K).astype(np.float32)
    out = kernel(params, 256, 256)
    print("kernel ran, out", out.shape, out.dtype, np.isnan(out).sum())
